# revision 13
# baseline (speedup 1.0000x reference)
"""ARIMA negative log-likelihood on 8 Trainium2 NeuronCores.

Strategy: the MA recurrence e_t = z_t - sum_j theta_j e_{t-j-1} is a linear
constant-coefficient recurrence, so e = T^{-1} z with T unit-diagonal banded
lower-triangular Toeplitz. T^{-1} is exactly lower-triangular Toeplitz with
the IIR impulse-response coefficients psi (psi_0=1, psi_k=-sum_j th_j
psi_{k-j}), and composing with the AR part gives e = C y where C is
lower-triangular Toeplitz with c = conv(psi, [1, -phi]). With theta in
[0, 0.1]^4 the taps decay at least like 0.72^k, so taps >= 128 are < 1e-19
and the scan collapses into a <=256-tap causal FIR along time: per 128-step
time tile, e_tile = W0^T @ y_tile + W1^T @ y_prev_tile — two TensorEngine
matmuls per tile. sum(e^2) is accumulated per partition by the Vector/Scalar
engines straight out of PSUM.

Sharding: data-parallel over batch B=64 -> 8 cores x 8 batches (the scan is
only over time, so shards are independent; only the final scalar reduces).
Host work is O(L^2 + cores*128*32): filter taps, Toeplitz fill, final sum.
"""
import sys
import types
import numpy as np

if "/opt/trn_rl_repo" not in sys.path:
    sys.path.insert(0, "/opt/trn_rl_repo")

B, S, C = 64, 2048, 128
P, Q = 4, 4
N_CORES = 8
B_LOC = B // N_CORES          # 8 batches per core
TT = 128                      # time-tile size (partition dim)
NT = S // TT                  # 16 time tiles
BG = 4                        # batches per matmul group (N = BG*C = 512)
NG = B_LOC // BG              # 2 batch groups per core
MT = 4                        # time tiles per DMA (1 MB per DMA)
NM = NT // MT                 # 4 DMA chunks per (core, batch-group)
NSLOT = NG * NT               # 32 square-reduce slots per core

NQUAD = NSLOT // 4            # square-reduce batches of 4 PSUM banks (FD=2048)

# quad assignment between Scalar (ACT, reads PSUM directly) and Vector
# (DVE, copy->bf16 then reduce); ACT takes more in bf16 mode since DVE
# also does the f32->bf16 input casts.
_DVE_QUADS_BF16 = frozenset()   # TTR lowers to an ISA blob the pinned
_DVE_QUADS_F32 = frozenset()    # walrus rejects; keep squares on ACT

MODE = "bf16"  # "bf16" | "f32r" | "f32"

_CACHE = {}


def _install_shims():
    """Two environment shims, idempotent.

    1. The pinned walrus rejects instructions carrying >1 sync wait
       ("Too many sync wait commands" on the Tile tail Drain). Split such
       instructions at the BIR-JSON level into single-wait Drains.
    2. antenv.axon_hooks is absent in this image; provide it so
       run_bass_kernel_spmd(trace=True) can find the NTFF profile hook.
    """
    import json
    import concourse.bass2jax as b2j
    import concourse.bass_utils as bu

    if not getattr(b2j, "_multiwait_patch", False):
        def _split_multiwait(bir_json):
            d = json.loads(bir_json)
            changed = False
            uniq = [0]
            for fn in d.get("functions", []):
                for blk in fn.get("blocks", []):
                    new_insts = []
                    for ins in blk.get("instructions", []):
                        si = ins.get("sync_info") or {}
                        waits = si.get("on_wait") or []
                        if len(waits) > 1:
                            changed = True
                            for w in waits[:-1]:
                                uniq[0] += 1
                                new_insts.append({
                                    "name": f"{ins.get('name', 'I')}-w{uniq[0]}",
                                    "opcode": "Drain",
                                    "engine": ins.get("engine"),
                                    "ins": [], "outs": [],
                                    "is_reset_sema": False,
                                    "debug": ins.get("debug", 0),
                                    "sync_info": {"on_update": [], "on_wait": [w]},
                                })
                            si["on_wait"] = [waits[-1]]
                        new_insts.append(ins)
                    blk["instructions"] = new_insts
            return json.dumps(d).encode() if changed else bir_json

        _orig_compile = bu.compile_bir_kernel

        def _patched(bir_json, *a, **k):
            return _orig_compile(_split_multiwait(bir_json), *a, **k)

        b2j.compile_bir_kernel = _patched
        b2j._multiwait_patch = True

    if "antenv.axon_hooks" not in sys.modules:
        mod = types.ModuleType("antenv.axon_hooks")
        mod._hook = None
        def _set(h, _m=mod):
            _m._hook = h
        def _get(_m=mod):
            return _m._hook
        mod.set_axon_ntff_profile_hook = _set
        mod.get_axon_ntff_profile_hook = _get
        sys.modules["antenv.axon_hooks"] = mod
        try:
            from trn_agent_boot.trn_boot import _ntff_profile_via_ctypes
            hook = _ntff_profile_via_ctypes("/opt/axon/libaxon_pjrt.so")
            if hook is not None:
                _set(hook)
        except Exception:
            pass


def _filter_coeffs(phi, theta, L=256):
    """Taps c_k of the composite causal FIR, float64 host-side."""
    a = np.zeros(L, dtype=np.float64)
    a[0] = 1.0
    a[1:1 + P] = -np.asarray(phi, dtype=np.float64)
    psi = np.zeros(L, dtype=np.float64)
    psi[0] = 1.0
    th = np.asarray(theta, dtype=np.float64)
    for k in range(1, L):
        lo = max(0, k - Q)
        # psi_k = -sum_{j=1..Q} th_j psi_{k-j}
        psi[k] = -np.dot(th[:k - lo][::-1], psi[lo:k])
    return np.convolve(psi, a)[:L]


def _toeplitz_mats(c):
    """lhsT layouts: w0[j,i] = c[i-j]; w1[j,i] = c[i+128-j] (j = contraction)."""
    idx = np.arange(TT)
    k0 = idx[None, :] - idx[:, None]          # i - j
    w0 = np.where((k0 >= 0) & (k0 < 2 * TT), c[np.clip(k0, 0, 2 * TT - 1)], 0.0)
    k1 = idx[None, :] + TT - idx[:, None]     # i + 128 - j
    w1 = np.where(k1 < 2 * TT, c[np.clip(k1, 0, 2 * TT - 1)], 0.0)
    return np.ascontiguousarray(w0, dtype=np.float32), np.ascontiguousarray(w1, dtype=np.float32)


def _build_nc(mode):
    import concourse.bass as bass
    import concourse.mybir as mybir
    import concourse.tile as tile

    f32 = mybir.dt.float32
    bf16 = mybir.dt.bfloat16
    f32r = mybir.dt.float32r

    nc = bass.Bass()
    y = nc.declare_dram_parameter("y", [B_LOC, S, C], f32, isOutput=False)
    w0d = nc.declare_dram_parameter("w0", [TT, TT], f32, isOutput=False)
    w1d = nc.declare_dram_parameter("w1", [TT, TT], f32, isOutput=False)
    out = nc.declare_dram_parameter("partials", [TT, 2 * NQUAD], f32, isOutput=True)

    # DRAM view: b (m n p) c -> m n p b c   (p = partition dim)
    yr = y.rearrange("b (m n p) c -> m n p b c", m=NM, n=MT, p=TT)

    with tile.TileContext(nc) as tc:
        with (
            tc.tile_pool(name="w", bufs=1) as wpool,
            tc.tile_pool(name="y32", bufs=3) as y32pool,
            tc.tile_pool(name="ybf", bufs=3) as ybfpool,
            tc.tile_pool(name="psum", bufs=2, space="PSUM") as psumpool,
            tc.tile_pool(name="scratch", bufs=2) as scpool,
            tc.tile_pool(name="stats", bufs=1) as stpool,
        ):
            w0s = wpool.tile([TT, TT], f32, tag="w0")
            nc.sync.dma_start(w0s[:], w0d[:])
            w1s = wpool.tile([TT, TT], f32, tag="w1")
            nc.sync.dma_start(w1s[:], w1d[:])
            if mode == "bf16":
                w0b = wpool.tile([TT, TT], bf16, tag="w0b")
                nc.scalar.copy(w0b[:], w0s[:])
                w1b = wpool.tile([TT, TT], bf16, tag="w1b")
                nc.scalar.copy(w1b[:], w1s[:])
                lhs0, lhs1 = w0b[:], w1b[:]
            elif mode == "f32r":
                lhs0, lhs1 = w0s[:].bitcast(f32r), w1s[:].bitcast(f32r)
            else:
                lhs0, lhs1 = w0s[:], w1s[:]

            dve_quads = _DVE_QUADS_BF16 if mode == "bf16" else _DVE_QUADS_F32
            stats_dve = (stpool.tile([TT, NQUAD], f32, tag="sd")
                         if dve_quads else None)
            stats_act = stpool.tile([TT, NQUAD], f32, tag="sa")
            n_dve = 0
            n_act = 0

            prev = [None] * NG  # per batch-group: previous time tile's rhs AP
            ps4 = None
            for m in range(NM):
                y32 = y32pool.tile([TT, MT, B_LOC, C], f32, tag="y32")
                for n in range(MT):
                    nc.gpsimd.dma_start(y32[:, n], yr[m, n])
                if mode == "bf16":
                    src = ybfpool.tile([TT, MT, B_LOC, C], bf16, tag="ybf")
                    nc.vector.tensor_copy(src[:], y32[:])
                else:
                    src = y32
                for n in range(MT):
                    for g in range(NG):
                        t = (m * MT + n) * NG + g
                        q, j = divmod(t, 4)
                        rhs = src[:, n, g * BG:(g + 1) * BG, :]
                        if mode == "f32r":
                            rhs = rhs.bitcast(f32r)
                        if j == 0:
                            ps4 = psumpool.tile([TT, 4, BG * C], f32, tag="ps")
                        nc.tensor.matmul(ps4[:, j], lhs0, rhs,
                                         start=True, stop=(prev[g] is None))
                        if prev[g] is not None:
                            nc.tensor.matmul(ps4[:, j], lhs1, prev[g],
                                             start=False, stop=True)
                        prev[g] = rhs
                        if j == 3:
                            if q in dve_quads:
                                sc = scpool.tile([TT, 4, BG * C], bf16,
                                                 tag="sc_dve")
                                nc.vector.tensor_copy(sc[:], ps4[:])
                                junk = scpool.tile([TT, 4, BG * C], bf16,
                                                   tag="junk_dve")
                                nc.vector.tensor_tensor_reduce(
                                    out=junk[:], in0=sc[:], in1=sc[:],
                                    scale=1.0, scalar=0.0,
                                    op0=mybir.AluOpType.mult,
                                    op1=mybir.AluOpType.add,
                                    accum_out=stats_dve[:, n_dve:n_dve + 1],
                                )
                                n_dve += 1
                            else:
                                sc = scpool.tile([TT, 4, BG * C], bf16,
                                                 tag="sc_act")
                                nc.scalar.activation(
                                    sc[:], ps4[:],
                                    mybir.ActivationFunctionType.Square,
                                    accum_out=stats_act[:, n_act:n_act + 1],
                                )
                                n_act += 1

            if n_dve:
                nc.sync.dma_start(out[:, 0:n_dve], stats_dve[:, 0:n_dve])
            if n_act:
                nc.sync.dma_start(out[:, NQUAD:NQUAD + n_act],
                                  stats_act[:, 0:n_act])

    return nc


def _run(y, phi, theta, sigma2, mode=MODE, trace=False):
    _install_shims()
    from concourse.bass_utils import run_bass_kernel_spmd

    if mode not in _CACHE:
        _CACHE[mode] = _build_nc(mode)
    nc = _CACHE[mode]

    c = _filter_coeffs(phi, theta)
    w0, w1 = _toeplitz_mats(c)

    y = np.ascontiguousarray(y, dtype=np.float32)
    in_maps = [
        {"y": y[i * B_LOC:(i + 1) * B_LOC], "w0": w0, "w1": w1}
        for i in range(N_CORES)
    ]
    res = run_bass_kernel_spmd(nc, in_maps, core_ids=list(range(N_CORES)),
                               trace=trace)

    total = np.float64(0.0)
    for r in res.results:
        total += r["partials"].astype(np.float64).sum()

    s2 = np.float64(np.asarray(sigma2).reshape(-1)[0])
    nll = 0.5 * S * np.log(2.0 * np.pi * s2) + 0.5 * total / s2
    return np.array([nll], dtype=np.float32), res


def kernel(y, phi, theta, sigma2):
    out, _ = _run(y, phi, theta, sigma2, mode=MODE, trace=False)
    return out
